# revision 4
# baseline (speedup 1.0000x reference)
"""EMA recurrence kernel for Trainium2 (8 NeuronCores, Bass/Tile).

Computes a_t = w * x_t + (1 - w) * a_{t-1} over inputs [B=32, T=8192, C=128],
initial_state [B, C], weights [C] -> output [B, T, C].

Strategy:
  - Pure data parallelism: batch dim sharded 4-per-core across 8 cores.
  - Per core, per batch: time is processed in chunks of 1024 steps.
    HBM layout is [T, C] (time-major); the scan needs [C(part), T(free)].
    * DMA in natural layout (contiguous, full bandwidth)
    * PE (tensor engine) transposes 128x128 blocks into PSUM
    * ACT evacuates PSUM -> SBUF with fused per-channel scale (w * x)
    * DVE tensor_tensor_scan runs the recurrence along the free (time) dim,
      chained across chunks via initial=prev[:, -1:]
    * PE transposes the result back to natural layout (PSUM)
    * ACT evacuates PSUM -> SBUF, DMA out.
"""

import sys

if "/opt/trn_rl_repo" not in sys.path:
    sys.path.insert(0, "/opt/trn_rl_repo")

import numpy as np

B, T, C = 32, 8192, 128
NCORES = 8
BL = B // NCORES  # batches per core
CHUNK = 1024      # time steps per scan chunk
NCH = T // CHUNK  # chunks per batch
NBLK = CHUNK // 128  # 128-blocks per chunk
NBLK_T = T // 128    # 128-blocks per batch

_NC_CACHE = None


def build_bass():
    global _NC_CACHE
    if _NC_CACHE is not None:
        return _NC_CACHE

    import concourse.bacc as bacc
    import concourse.mybir as mybir
    import concourse.tile as tile

    f32 = mybir.dt.float32
    AF = mybir.ActivationFunctionType
    ALU = mybir.AluOpType

    nc = bacc.Bacc("TRN2", target_bir_lowering=False, debug=False)
    x = nc.dram_tensor("x", [BL, T, C], f32, kind="ExternalInput").ap()
    s0T = nc.dram_tensor("s0T", [C, BL], f32, kind="ExternalInput").ap()
    cdec = nc.dram_tensor("cdec", [C, CHUNK], f32, kind="ExternalInput").ap()
    wcol = nc.dram_tensor("wcol", [C, 1], f32, kind="ExternalInput").ap()
    ident = nc.dram_tensor("ident", [128, 128], f32, kind="ExternalInput").ap()
    y = nc.dram_tensor("y", [BL, T, C], f32, kind="ExternalOutput").ap()

    with tile.TileContext(nc) as tc:
        with (
            tc.tile_pool(name="const", bufs=1) as cpool,
            tc.tile_pool(name="io", bufs=2) as io_pool,
            tc.tile_pool(name="work", bufs=3) as wpool,
            tc.tile_pool(name="pin", bufs=2, space="PSUM") as pin_pool,
            tc.tile_pool(name="pout", bufs=2, space="PSUM") as pout_pool,
        ):
            s0T_t = cpool.tile([C, BL], f32, name="s0T_t")
            nc.sync.dma_start(s0T_t[:], s0T[:])
            cdec_t = cpool.tile([C, CHUNK], f32, name="cdec_t")
            nc.sync.dma_start(cdec_t[:], cdec[:])
            wcol_t = cpool.tile([C, 1], f32, name="wcol_t")
            nc.sync.dma_start(wcol_t[:], wcol[:])
            ident_t = cpool.tile([128, 128], f32, name="ident_t")
            nc.sync.dma_start(ident_t[:], ident[:])

            for b in range(BL):
                # natural-layout input: [t_lo(part), block, c]
                xin = io_pool.tile([128, NBLK_T, C], f32, name="xin", tag="xin")
                nc.sync.dma_start(
                    xin[:], x[b].rearrange("(n p) c -> p n c", p=128)
                )
                yout = io_pool.tile([128, NBLK_T, C], f32, name="yout", tag="yout")

                prev = None
                for k in range(NCH):
                    # transpose chunk into [c(part), t(free)] in PSUM
                    xps = pin_pool.tile([C, NBLK, 128], f32, name="xps", tag="xps")
                    for j in range(NBLK):
                        nc.tensor.transpose(
                            xps[:, j, :], xin[:, k * NBLK + j, :], ident_t[:]
                        )
                    # B = w * x^T  (per-partition scale), PSUM -> SBUF
                    bsb = wpool.tile([C, CHUNK], f32, name="bsb", tag="bsb")
                    nc.scalar.activation(
                        bsb[:],
                        xps.rearrange("p n c -> p (n c)"),
                        AF.Copy,
                        scale=wcol_t[:],
                    )
                    # a_t = (1-w) * a_{t-1} + B_t along free dim
                    asb = wpool.tile([C, CHUNK], f32, name="asb", tag="asb", bufs=4)
                    init = s0T_t[:, b : b + 1] if k == 0 else prev[:, CHUNK - 1 : CHUNK]
                    nc.vector.tensor_tensor_scan(
                        asb[:], cdec_t[:], bsb[:], init, op0=ALU.mult, op1=ALU.add
                    )
                    prev = asb
                    # transpose back to natural layout
                    yps = pout_pool.tile([128, NBLK, C], f32, name="yps", tag="yps")
                    for j in range(NBLK):
                        nc.tensor.transpose(
                            yps[:, j, :], asb[:, j * 128 : (j + 1) * 128], ident_t[:]
                        )
                    nc.scalar.activation(
                        yout[:, k * NBLK : (k + 1) * NBLK, :], yps[:], AF.Copy
                    )
                nc.sync.dma_start(
                    y[b].rearrange("(n p) c -> p n c", p=128), yout[:]
                )

    nc.compile()
    _NC_CACHE = nc
    return nc


def _in_maps(inputs, initial_state, weights):
    x = np.ascontiguousarray(np.asarray(inputs, dtype=np.float32))
    s0 = np.asarray(initial_state, dtype=np.float32)
    w = np.clip(np.asarray(weights, dtype=np.float32), 0.0, 1.0)
    c = (1.0 - w).astype(np.float32)

    cdec = np.ascontiguousarray(np.repeat(c[:, None], CHUNK, axis=1))
    wcol = np.ascontiguousarray(w[:, None])
    ident = np.eye(128, dtype=np.float32)

    maps = []
    for i in range(NCORES):
        maps.append(
            {
                "x": np.ascontiguousarray(x[i * BL : (i + 1) * BL]),
                "s0T": np.ascontiguousarray(s0[i * BL : (i + 1) * BL].T),
                "cdec": cdec,
                "wcol": wcol,
                "ident": ident,
            }
        )
    return maps


def _ensure_ntff_hook():
    """Shim antenv.axon_hooks (absent in this image) so trace=True works."""
    import types

    import antenv

    if not hasattr(antenv, "axon_hooks"):
        mod = types.ModuleType("antenv.axon_hooks")
        holder = [None]
        mod.set_axon_ntff_profile_hook = lambda h: holder.__setitem__(0, h)
        mod.get_axon_ntff_profile_hook = lambda: holder[0]
        sys.modules["antenv.axon_hooks"] = mod
        antenv.axon_hooks = mod
    from antenv.axon_hooks import (
        get_axon_ntff_profile_hook,
        set_axon_ntff_profile_hook,
    )

    if get_axon_ntff_profile_hook() is None:
        from trn_agent_boot.trn_boot import _ntff_profile_via_ctypes

        set_axon_ntff_profile_hook(
            _ntff_profile_via_ctypes("/opt/axon/libaxon_pjrt.so")
        )


def run(inputs, initial_state, weights, trace=False, **kw):
    from concourse import bass_utils

    if trace:
        _ensure_ntff_hook()
    nc = build_bass()
    maps = _in_maps(inputs, initial_state, weights)
    res = bass_utils.run_bass_kernel_spmd(
        nc, maps, core_ids=list(range(NCORES)), trace=trace, **kw
    )
    out = np.concatenate([r["y"] for r in res.results], axis=0)
    return out, res


def kernel(inputs, initial_state, weights):
    out, _ = run(inputs, initial_state, weights)
    return out
